# revision 19
# baseline (speedup 1.0000x reference)
"""Trainium2 Bass kernel for nn_BaconAdditionReasoner (histogram_binning).

Math (per batch row):
    P1 = soft_perm(W1), P2 = soft_perm(W2)           (host, 10x10)
    l1 = p1 @ P1.T, l2 = p2 @ P2.T
    u = log(1-l1), v = log(1-l2)
    logprod[k] = sum_{i+j=k} max(u_i, v_j)
              = sum_{i+j=k} u_i + sum_{i+j=k} relu(v_j - u_i)
    e = exp(logprod);  out_k = (e_k - 1) / (sum_k e_k - 19)

Device dataflow (data parallel over 8 cores, 32768 rows/core):
  Front is feature-major: 4 bands of 20 feature rows at 32-aligned
  partitions (PE tile_position needs 32-aligned moving bases), batch on the
  free dim, in half-supertiles of 512 cols (2048 rows); processed in PAIRS
  (1024 cols) so Ln / Exp / the normalization tail amortize their
  per-instruction SBUF/PSUM access cost.
  - L matmul: blockdiag [116->116] f16 matmul per half
  - Ln (ACT): uv = log(1 - l) -> f16, one instr per pair
  - D matmuls: per band, [20->110] f16 pair-diff expansion in PSUM
    (v_j - u_i pairs + -u passthrough rows); 4 PSUM bufs = all bands in
    flight, so one slow relu never starves the PE
  - relu: split across ACT / DVE / Pool per band tile (tunable scheme)
  - A-flip matmuls: per 128-col block, kt [110, 128] is the STATIONARY
    operand (LDWEIGHTS is free) and a [110, 19] +-1 matrix streams as the
    moving operand -> 19 cycles/block; batch-major logprod in PSUM
  - Exp (ACT) -> f32; 19-group reduce / recip / (e-1)*r on DVE per pair
  - output dumped partition-major [128, 4864] f16; host de-interleaves.

All HBM I/O and matmul moving operands are f16 (validated on the real
input distribution: max rel err ~2.7e-3 vs the 2e-2 gate).
"""

import numpy as np

# ---------------------------------------------------------------- constants
B = 262144
NCORES = 8
BC = B // NCORES            # 32768 rows per core
F = 512                     # batch columns per half-supertile (per band)
NB = 4                      # bands (32-aligned partition offsets)
ROWS_H = F * NB             # 2048 rows per half
NH = BC // ROWS_H           # 16 halves per core
NP = NH // 2                # 8 pairs
NCOLS = NH * F              # 8192 columns in pc
KCH = 19 * (ROWS_H // 128)  # 304 output cols per half
KC = 2 * KCH                # 608 per pair
OCOLS = KC * NP             # 4864 output cols

# wk (constants, f16 [128, 256]) column layout
WL0, WL1 = 0, 116           # L blockdiag lhsT [116, 116]
WD0, WD1 = 116, 226         # D pair lhsT [20, 110] replicated per band
WA0, WA1 = 226, 245         # A-flip moving [110, 19]
WKC = 256                   # padded so DMA elem = 512 B

# relu engine per band tile (8 tiles per pair = 2 halves x 4 bands), each a
# list of (engine, c0, c1) over [0, F).
# engines: "A" = ACT (scalar), "D" = DVE (vector), "P" = Pool (gpsimd)
RELU_SCHEME = [
    [("A", 0, F)], [("D", 0, F)], [("P", 0, F)], [("P", 0, F)],
    [("A", 0, F)], [("D", 0, F)], [("D", 0, 128), ("P", 128, F)], [("P", 0, F)],
]


def _soft_perm_np(W: np.ndarray) -> np.ndarray:
    W = W.astype(np.float32)
    lo = W.min(axis=1, keepdims=True)
    hi = W.max(axis=1, keepdims=True)
    Wn = (W - lo) / (hi - lo + np.float32(1e-8))
    return Wn / (Wn.sum(axis=1, keepdims=True) + np.float32(1e-8))


def _build_wk(P1n: np.ndarray, P2n: np.ndarray) -> np.ndarray:
    wk = np.zeros((128, WKC), dtype=np.float32)
    # --- L: lhsT[32q+d, 32q+e] = PP[e, d], PP = blockdiag(P1n, P2n)
    for q in range(NB):
        r = 32 * q
        wk[r : r + 10, r : r + 10] = P1n.T
        wk[r + 10 : r + 20, r + 10 : r + 20] = P2n.T
    # --- D: [20, 110]: pair col 10i+j gets v_j - u_i; col 100+e gets -u_e
    d = np.zeros((20, 110), dtype=np.float32)
    for i in range(10):
        for j in range(10):
            d[i, 10 * i + j] = -1.0
            d[10 + j, 10 * i + j] = 1.0
    for e in range(10):
        d[e, 100 + e] = -1.0
    for q in range(NB):
        wk[32 * q : 32 * q + 20, WD0:WD1] = d
    # --- A-flip moving [110, 19]: pair rows +1 at k=i+j; passthrough rows
    #     (-u values) -1 for k in [e, e+9]
    a = np.zeros((110, 19), dtype=np.float32)
    for i in range(10):
        for j in range(10):
            a[10 * i + j, i + j] = 1.0
    for e in range(10):
        a[100 + e, e : e + 10] = -1.0
    wk[0:110, WA0:WA1] = a
    return wk.astype(np.float16)


def _build_pc(p1c: np.ndarray, p2c: np.ndarray) -> np.ndarray:
    """[BC,10]x2 -> pc [116, NCOLS] f16: row 32q+e = feature e (u: e<10,
    v: 10<=e<20) of band q; col F*g+f = batch row ROWS_H*g + F*q + f."""
    pc = np.zeros((116, NCOLS), dtype=np.float16)
    x1 = p1c.reshape(NH, NB, F, 10)     # [g, q, f, d]
    x2 = p2c.reshape(NH, NB, F, 10)
    for q in range(NB):
        pc[32 * q : 32 * q + 10, :] = (
            x1[:, q].transpose(2, 0, 1).reshape(10, NCOLS).astype(np.float16)
        )
        pc[32 * q + 10 : 32 * q + 20, :] = (
            x2[:, q].transpose(2, 0, 1).reshape(10, NCOLS).astype(np.float16)
        )
    return pc


def _unpack_yraw(yraw: np.ndarray) -> np.ndarray:
    """yraw [128, OCOLS] f16 -> y [BC, 19] f32.
    yraw[p, KC*t + 19*(16h+4q+b) + k] = y[4096t+2048h+512q+128b+p, k]."""
    t = yraw.reshape(128, NP, 2, NB, 4, 19).transpose(1, 2, 3, 4, 0, 5)
    return np.ascontiguousarray(t.reshape(BC, 19).astype(np.float32))


def _patch_act_tables():
    """Force Ln/Exp/Relu to resolve to the single set containing all three
    (natural_log_exp_and_others) so the activation table is loaded once."""
    import concourse.bacc as bacc
    from concourse import mybir

    if getattr(bacc, "_act_tables_patched", False):
        return
    orig = bacc.get_activation_tables
    AF = mybir.ActivationFunctionType
    shared = {AF.Ln, AF.Exp, AF.Relu}

    def patched(arch):
        tabs = orig(arch)
        if "natural_log_exp_and_others" in tabs:
            for name, funcs in tabs.items():
                if name != "natural_log_exp_and_others":
                    tabs[name] = set(funcs) - shared
        return tabs

    bacc.get_activation_tables = patched
    bacc._act_tables_patched = True


def build_bass():
    import concourse.bass as bass
    import concourse.bacc as bacc
    import concourse.tile as tile
    from concourse import mybir

    _patch_act_tables()
    f32 = mybir.dt.float32
    f16 = mybir.dt.float16
    AF = mybir.ActivationFunctionType
    ALU = mybir.AluOpType

    nc = bacc.Bacc("TRN2", target_bir_lowering=False)

    pc_d = nc.dram_tensor("pc", [116, NCOLS], f16, kind="ExternalInput")
    wk_d = nc.dram_tensor("wk", [128, WKC], f16, kind="ExternalInput")
    y_d = nc.dram_tensor("yraw", [128, OCOLS], f16, kind="ExternalOutput")

    with tile.TileContext(nc) as tc:
        with (
            tc.tile_pool(name="singles", bufs=1) as singles,
            tc.tile_pool(name="uv", bufs=2) as uv_p,
            tc.tile_pool(name="kt", bufs=4) as kt_p,
            tc.tile_pool(name="ee", bufs=2) as ee_p,
            tc.tile_pool(name="ss", bufs=2) as ss_p,
            tc.tile_pool(name="rr", bufs=2) as rr_p,
            tc.tile_pool(name="psL", bufs=1, space="PSUM") as psL,
            tc.tile_pool(name="psD", bufs=4, space="PSUM") as psD,
            tc.tile_pool(name="psA", bufs=1, space="PSUM") as psA,
        ):
            oo = singles.tile([128, OCOLS], f16)

            # prefetch the whole input up front (16 KB/partition); first chunk
            # is one pair so compute starts as early as possible
            pcc = singles.tile([116, NCOLS], f16)
            nc.sync.dma_start(pcc[:, 0:1024], pc_d[:, 0:1024])
            wk = singles.tile([128, WKC], f16)
            nc.sync.dma_start(wk[:, :], wk_d[:, :])
            for c0, c1 in [(1024, 2048), (2048, 4096), (4096, 8192)]:
                nc.sync.dma_start(pcc[:, c0:c1], pc_d[:, c0:c1])

            # while the first DMAs land: preload the Ln/Exp/Relu activation
            # table so the first real Ln doesn't eat the 1.3us table load
            wz = singles.tile([20, 8], f16)
            nc.vector.memset(wz[:, :], 0.0)
            wz2 = singles.tile([128, 8], f16)
            nc.scalar.activation(wz2[0:1, 0:1], wz[0:1, 0:1], AF.Ln)

            def emit_relu(scheme, dp):
                kt = kt_p.tile([110, F], f16)
                for eng, a0, a1 in scheme:
                    if eng == "A":
                        nc.scalar.activation(kt[:, a0:a1], dp[:, a0:a1], AF.Relu)
                    elif eng == "D":
                        nc.vector.tensor_scalar(
                            kt[:, a0:a1], dp[:, a0:a1], 0.0, None, op0=ALU.max
                        )
                    else:
                        nc.gpsimd.tensor_scalar(
                            kt[:, a0:a1], dp[:, a0:a1], 0.0, None, op0=ALU.max
                        )
                return kt

            def emit_tail(pt):
                # normalization tail over oo cols [o0, o0+w); dma [d0, d1)
                e32, o0, w, dmarange = pt
                nb = w // 19
                ev = e32[:, 0:w].rearrange("p (b k) -> p b k", b=nb, k=19)
                s32 = ss_p.tile([128, nb], f32)
                nc.vector.tensor_reduce(
                    s32[:, :], ev, axis=mybir.AxisListType.X, op=ALU.add
                )
                sm = ss_p.tile([128, nb], f32)
                nc.vector.tensor_scalar(
                    sm[:, :], s32[:, :], -19.0, None, op0=ALU.add
                )
                r32 = rr_p.tile([128, nb], f32)
                nc.vector.reciprocal(r32[:, :], sm[:, :])
                # out = (e - 1) * r  ==  (1-e)/(19-sum(e)), f16
                ov = oo[:, o0 : o0 + w].rearrange("p (b k) -> p b k", b=nb, k=19)
                rb = r32[:, :].unsqueeze(-1).broadcast_to([128, nb, 19])
                nc.vector.scalar_tensor_tensor(
                    ov, ev, 1.0, rb, op0=ALU.subtract, op1=ALU.mult
                )
                if dmarange is not None:
                    d0, d1 = dmarange
                    nc.sync.dma_start(y_d[:, d0:d1], oo[:, d0:d1])

            def emit_front(t):
                # l = blockdiag(P1n, P2n) @ p ; one Ln per pair
                base = 2 * F * t
                lp = psL.tile([116, 2 * F], f32)
                for h in range(2):
                    nc.tensor.matmul(
                        lp[:, F * h : F * (h + 1)], wk[0:116, WL0:WL1],
                        pcc[0:116, base + F * h : base + F * (h + 1)],
                        start=True, stop=True,
                    )
                uvt = uv_p.tile([116, 2 * F], f16)
                nc.scalar.activation(
                    uvt[:, :], lp[:, :], AF.Ln, bias=1.0, scale=-1.0
                )
                return uvt

            pending = None
            uvt_next = emit_front(0)
            for t in range(NP):
                uvt = uvt_next
                last = t == NP - 1
                ap_t = psA.tile([128, KC], f32)
                for h in range(2):
                    uh = uvt[:, F * h : F * (h + 1)]
                    dps = []
                    for q in range(NB):
                        r = 32 * q
                        dp = psD.tile([110, F], f32)
                        nc.tensor.matmul(
                            dp[:, :], wk[r : r + 20, WD0:WD1], uh[r : r + 20, :],
                            start=True, stop=True, tile_position=(r, 0),
                        )
                        dps.append(dp)
                    kts = [
                        emit_relu(RELU_SCHEME[4 * h + q], dps[q])
                        for q in range(NB)
                    ]
                    # batch-major logprod: kt block stationary, [110,19] moving
                    for q in range(NB):
                        for b in range(F // 128):
                            blk = 16 * h + 4 * q + b
                            nc.tensor.matmul(
                                ap_t[:, 19 * blk : 19 * blk + 19],
                                kts[q][0:110, 128 * b : 128 * b + 128],
                                wk[0:110, WA0:WA1],
                                start=True, stop=True,
                            )
                    if last:
                        # final pair: per-half (then per-quarter) Exp +
                        # immediate tails + split DMAs to shorten the closing
                        # serial chain
                        if h == 0:
                            e32h = ee_p.tile([128, KCH], f32)
                            nc.scalar.activation(
                                e32h[:, :], ap_t[:, 0:KCH], AF.Exp
                            )
                            if pending is not None:
                                emit_tail(pending)
                                pending = None
                            o0 = KC * t
                            emit_tail((e32h, o0, KCH, (KC * (t - 1), o0 + KCH)))
                        else:
                            w = KCH // 2
                            for qq in range(2):
                                a0 = KCH + w * qq
                                e32q = ee_p.tile([128, w], f32)
                                nc.scalar.activation(
                                    e32q[:, :], ap_t[:, a0 : a0 + w], AF.Exp
                                )
                                o0 = KC * t + a0
                                emit_tail((e32q, o0, w, (o0, o0 + w)))

                if not last:
                    # hoist next pair's front so ACT starts Ln(t+1) while it
                    # waits for this pair's last A-flips
                    uvt_next = emit_front(t + 1)
                    # e = exp(logprod) (f32: e-1 cancellation needs mantissa)
                    e32 = ee_p.tile([128, KC], f32)
                    nc.scalar.activation(e32[:, :], ap_t[:, :], AF.Exp)
                    # defer this pair's tail until after the NEXT pair's grid
                    # so the DVE's in-order stream never lets the tail gate
                    # the next pair's relus
                    if pending is not None:
                        emit_tail(pending)
                    dma = None
                    if t % 2 == 1 and t != NP - 2:
                        dma = (KC * (t - 1), KC * (t + 1))
                    pending = (e32, KC * t, KC, dma)
    nc.compile()
    return nc


_NC_CACHE = None


def kernel(p1, p2, W1, W2):
    global _NC_CACHE
    from concourse.bass_utils import run_bass_kernel_spmd

    P1n = _soft_perm_np(np.asarray(W1))
    P2n = _soft_perm_np(np.asarray(W2))
    wk = _build_wk(P1n, P2n)
    p1 = np.ascontiguousarray(np.asarray(p1, dtype=np.float32))
    p2 = np.ascontiguousarray(np.asarray(p2, dtype=np.float32))

    in_maps = []
    for c in range(NCORES):
        sl = slice(c * BC, (c + 1) * BC)
        in_maps.append({"pc": _build_pc(p1[sl], p2[sl]), "wk": wk})

    if _NC_CACHE is None:
        _NC_CACHE = build_bass()
    res = run_bass_kernel_spmd(_NC_CACHE, in_maps, core_ids=list(range(NCORES)))
    out = np.concatenate(
        [_unpack_yraw(res.results[c]["yraw"]) for c in range(NCORES)], axis=0
    )
    return out
